# revision 18
# baseline (speedup 1.0000x reference)
"""CQAttention (BiDAF context-query attention) forward kernel for 8 Trainium2
NeuronCores.

Full inputs: context (64,128,1024) f32, question (64,128,128) f32, w (384,) f32.
Full output: (64, 512, 1024) f32.

Sharding: pure data parallel over batch — 8 batches per core, w replicated.

Math (per batch, X = context[b] (H,C), Y = question[b] (H,Q), w=(wq,wc,wcq)):
    Z   = wcq*Y + wc 1^T                      # (H,Q); wq term is softmax-invariant
    S^T = Z^T @ X                             # (Q,C) scores
    P   = exp(S^T)                            # unnormalized softmax numerators
    d   = rowsum(P); r = 1/d                  # softmax denominators (per q-row)
    A   = (diag(r) Y^T)^T @ P                 # = a^T                (H,C)
    tt  = P @ X^T                             # (Q,H)
    Bm  = (diag(r^2) tt)^T @ P                # = b^T = (s1 (s1^T c))^T  (H,C)
    out = [X; A; X*A; X*Bm]                   # (4H, C)

The run is HBM-bound (17.3 MB/core of DMA at ~400 GB/s ≈ 45 us floor + ~6.5 us
fixed framework preamble). Structure chosen to keep DMA saturated and the PE
stream gapless (so its clock ramps 1.2 -> 2.4 GHz):

- tt needs both P and X with the contraction dim (C) on partitions. Instead of
  16 PE transposes, 8 "combo" matmuls per batch compute, for each 128-chunk of
  C, X_chunk^T @ [I | Z] = [X^T_chunk | S_chunk] — the X^T chunk AND the
  scores in (C,Q) layout in one N=256 f32r pass. A second exp turns S_chunk
  into P^T directly. This removes the exp -> transpose serialization entirely:
  combo depends only on X and Z.
- 2-cycle software pipeline: cycle b runs {S, exp, denominators, combo, exp^T,
  XT copies} of batch b and {A, tt, B, output muls, output DMAs} of batch b-1,
  so every PE instruction depends only on previous-cycle products.
- All inputs prefetched up front (~12 us of DMA backlog); outputs (A, X*A,
  X*B) stream per batch. Copy/elementwise work is spread across ACT, DVE and
  Pool (Pool cannot touch PSUM, so it gets the SBUF-only X*A muls + Z).

All matmuls run in float32r (TF32-like, 1 cycle/row at N>=256). X/Y are DMA'd
as raw f32 bits into f32r tiles; engine-written f32r tiles (P, PT, XT, Z, tts)
are round-to-nearest by hardware. Elementwise consumers use .bitcast(f32).
"""

import os
import sys

import numpy as np

if "/opt/trn_rl_repo" not in sys.path:
    sys.path.insert(0, "/opt/trn_rl_repo")

B, H, C, Q = 64, 128, 1024, 128
NCORES = 8
BPC = B // NCORES  # batches per core


def _ensure_ntff_hook():
    """This container's `antenv` stub lacks `axon_hooks`, which
    bass_utils needs for NTFF profiling under axon (trace=True). Install
    a functional shadow module + register the ctypes-based hook."""
    import types

    try:
        from antenv.axon_hooks import get_axon_ntff_profile_hook  # noqa: F401

        return  # real module present
    except ImportError:
        pass
    try:
        import antenv

        mod = types.ModuleType("antenv.axon_hooks")
        _state = {"hook": None}

        def set_axon_ntff_profile_hook(h):
            _state["hook"] = h

        def get_axon_ntff_profile_hook():
            return _state["hook"]

        mod.set_axon_ntff_profile_hook = set_axon_ntff_profile_hook
        mod.get_axon_ntff_profile_hook = get_axon_ntff_profile_hook
        sys.modules["antenv.axon_hooks"] = mod
        antenv.axon_hooks = mod

        from trn_agent_boot.trn_boot import _ntff_profile_via_ctypes

        set_axon_ntff_profile_hook(
            _ntff_profile_via_ctypes("/opt/axon/libaxon_pjrt.so")
        )
    except Exception:
        pass  # profiling degrades; compute still works


_ensure_ntff_hook()

LAST_RESULTS = None
_NC = None


def _build():
    from contextlib import ExitStack

    import concourse.bacc as bacc
    import concourse.mybir as mybir
    import concourse.tile as tile
    from concourse import masks

    f32 = mybir.dt.float32
    f32r = mybir.dt.float32r
    bf16 = mybir.dt.bfloat16
    EXP = mybir.ActivationFunctionType.Exp
    IDENT = mybir.ActivationFunctionType.Identity
    MULT = mybir.AluOpType.mult
    ADD = mybir.AluOpType.add

    nc = bacc.Bacc(
        "TRN2", target_bir_lowering=False, debug=False, enable_asserts=False
    )
    ctx_t = nc.dram_tensor("context", (BPC, H, C), f32, kind="ExternalInput").ap()
    q_t = nc.dram_tensor("question", (BPC, H, Q), f32, kind="ExternalInput").ap()
    w_t = nc.dram_tensor("w", (3 * H,), f32, kind="ExternalInput").ap()
    # device writes blocks (A, X*A, X*B); block 0 == context is filled
    # host-side during unshard (pure passthrough of an input).
    out_t = nc.dram_tensor("out", (BPC, 3, H, C), f32, kind="ExternalOutput").ap()

    with tile.TileContext(nc) as tc, ExitStack() as ctx:
        const = ctx.enter_context(tc.tile_pool(name="const", bufs=1))
        xp = ctx.enter_context(tc.tile_pool(name="xp", bufs=BPC))
        yp = ctx.enter_context(tc.tile_pool(name="yp", bufs=BPC))
        pp = ctx.enter_context(tc.tile_pool(name="pp", bufs=2))
        op = ctx.enter_context(tc.tile_pool(name="op", bufs=3))
        ps = ctx.enter_context(tc.tile_pool(name="ps", bufs=6, space="PSUM"))
        ps2 = ctx.enter_context(tc.tile_pool(name="ps2", bufs=2, space="PSUM"))

        ident = const.tile([128, 128], f32, tag="ident")
        masks.make_identity(nc, ident[:])
        identr = const.tile([128, 128], f32r, tag="identr")
        nc.vector.tensor_copy(identr[:], ident[:])

        # w arrives as one contiguous (1,384) row (cheap single-descriptor
        # DMA); the (128,1) columns are produced by K=1 PE matmuls against
        # identity — avoids two slow 128-descriptor scatter DMAs at startup.
        w_row = const.tile([1, 3 * H], f32r, tag="w_row")
        nc.sync.dma_start(w_row[:], w_t.unsqueeze(0).bitcast(f32r))

        # Prefetch ALL inputs up front: ~12 us of guaranteed DMA backlog
        # while the compute pipeline fills. Batch 0's X is split so S can
        # start on the first half sooner.
        Xs = [xp.tile([H, C], f32r, tag="X", name=f"X{i}") for i in range(BPC)]
        Yall = yp.tile([H, BPC * Q], f32r, tag="Y")
        Ys = [Yall[:, b * Q : (b + 1) * Q] for b in range(BPC)]
        nc.sync.dma_start(Xs[0][:, 0:512], ctx_t[0, :, 0:512].bitcast(f32r))
        # all 8 Y's in one trigger (0.5 MB) — input triggers beyond the DMA
        # semaphore pool (8) gate on earlier completions, so fewer is faster
        nc.sync.dma_start(
            Yall[:].rearrange("h (b q) -> h b q", b=BPC),
            q_t[:].transpose([1, 0, 2]).bitcast(f32r),
        )
        nc.sync.dma_start(Xs[0][:, 512:C], ctx_t[0, :, 512:C].bitcast(f32r))
        for b in range(1, BPC):
            nc.sync.dma_start(Xs[b][:], ctx_t[b].bitcast(f32r))

        wps = ps.tile([128, 512], f32, tag="ps")
        nc.tensor.matmul(
            wps[:, 0:128], w_row[0:1, H : 2 * H], identr[0:1, 0:128],
            start=True, stop=True,
        )
        nc.tensor.matmul(
            wps[:, 128:256], w_row[0:1, 2 * H : 3 * H], identr[0:1, 0:128],
            start=True, stop=True,
        )
        wc = const.tile([128, 1], f32, tag="wc")
        wcq = const.tile([128, 1], f32, tag="wcq")
        nc.vector.tensor_copy(wc[:], wps[:, 0:1])
        nc.vector.tensor_copy(wcq[:], wps[:, 128:129])

        # Z = wcq*Y + wc, computed on Pool one cycle ahead so S never waits.
        zq0 = const.tile([H, Q], f32r, tag="zq0")
        zq1 = const.tile([H, Q], f32r, tag="zq1")
        zqs = [zq0, zq1]

        def make_Z(b):
            nc.gpsimd.tensor_scalar(
                zqs[b % 2][:], Ys[b].bitcast(f32), wcq[:], wc[:],
                MULT, ADD,
            )

        make_Z(0)

        def front_a(b):
            # S matmuls + first exp + yt: the PE ops every same-cycle ACT/DVE
            # dep hangs off, so they lead the cycle on all queues.
            Zt = zqs[b % 2][:]
            P = pp.tile([Q, C], f32r, tag="P")
            dh = pp.tile([Q, 2], f32, tag="dh")
            Shs = []
            for j in range(2):
                Sh = ps.tile([Q, 512], f32, tag="ps")
                nc.tensor.matmul(
                    Sh[:], Zt, Xs[b][:, j * 512 : (j + 1) * 512],
                    start=True, stop=True,
                )
                Shs.append(Sh)
            nc.scalar.activation(
                P[:, 0:512], Shs[0][:], EXP, accum_out=dh[:, 0:1]
            )
            yt = ps2.tile([128, 256], f32, tag="tt")
            nc.tensor.transpose(yt[:, 0:128].bitcast(f32r), Ys[b], identr[:])
            # XT: [pad | X^T] in bf16 for the tt matmuls — the pad block
            # keeps every N=256 tt window on initialized data (cols 0:128 of
            # each window accumulate junk, never read). YTs: diag(r) Y^T,
            # the A-matmul stationary operand.
            XT = pp.tile([128, 128 + C], bf16, tag="XT")
            nc.gpsimd.memset(XT[:, 0:128], 0)
            YTs = pp.tile([Q, H], f32r, tag="YTs")
            return dict(b=b, P=P, dh=dh, Shs=Shs, yt=yt, XT=XT, YTs=YTs)

        def front_b(b, st):
            # second exp (after back's A-copies on ACT)
            P, dh, Shs = st["P"], st["dh"], st["Shs"]
            nc.scalar.activation(
                P[:, 512:C], Shs[1][:], EXP, accum_out=dh[:, 1:2]
            )

        def front_c(b, st):
            # softmax denominators: dsum on Pool (SBUF-only), reciprocal on
            # DVE (only engine with it), YTs on DVE. r2 is computed late on
            # Pool — its consumer (tts) runs next cycle.
            dh, yt = st["dh"], st["yt"]
            dsum = pp.tile([Q, 1], f32, tag="dsum")
            nc.vector.tensor_add(dsum[:], dh[:, 0:1], dh[:, 1:2])
            rr = pp.tile([Q, 1], f32, tag="rr")
            nc.vector.reciprocal(rr[:], dsum[:])
            nc.vector.tensor_scalar_mul(st["YTs"][:], yt[:, 0:128], rr[:])
            st.update(rr=rr)

        def front_d(b, st):
            r2 = pp.tile([Q, 1], f32, tag="r2")
            nc.gpsimd.tensor_mul(r2[:], st["rr"][:], st["rr"][:])
            if b + 1 < BPC:
                make_Z(b + 1)
            st.update(r2=r2)

        def mid_a(b, st):
            # PE transposes of P and X into PSUM (streamed, ~85 ns each).
            # pt0-3 gate on exp0 only; xt chunks are dep-free fillers; the
            # first XT copy (DVE) drains xtps0 early for the PSUM ring.
            X, P, XT = Xs[b], st["P"], st["XT"]
            PT = pp.tile([128, C], bf16, tag="PT")
            ptps0 = ps.tile([128, 512], f32, tag="ps")
            for k in range(4):
                nc.tensor.transpose(
                    ptps0[:, k * 128 : (k + 1) * 128].bitcast(f32r),
                    P[:, k * 128 : (k + 1) * 128],
                    identr[:],
                )
            xtps = []
            for g in range(2):
                xg = ps.tile([128, 512], f32, tag="ps")
                for k in range(4):
                    c0 = g * 4 + k
                    nc.tensor.transpose(
                        xg[:, k * 128 : (k + 1) * 128].bitcast(f32r),
                        X[:, c0 * 128 : (c0 + 1) * 128],
                        identr[:],
                    )
                xtps.append(xg)
                if g == 0:
                    nc.vector.tensor_copy(XT[:, 128:640], xg[:])
            ptps1 = ps.tile([128, 512], f32, tag="ps")
            for k in range(4):
                nc.tensor.transpose(
                    ptps1[:, k * 128 : (k + 1) * 128].bitcast(f32r),
                    P[:, 512 + k * 128 : 512 + (k + 1) * 128],
                    identr[:],
                )
            st.update(PT=PT, ptps=(ptps0, ptps1), xtps1=xtps[1])

        def mid_b(b, st):
            XT, PT = st["XT"], st["PT"]
            ptps0, ptps1 = st["ptps"]
            nc.vector.tensor_copy(XT[:, 640 : 128 + C], st["xtps1"][:])
            nc.scalar.copy(PT[:, 0:512], ptps0[:])
            nc.scalar.copy(PT[:, 512:C], ptps1[:])

        def back_a(st):
            # Deferred A-section for batch b-1: A matmuls lead (all deps are
            # previous-cycle), copies/muls/DMAs follow on early queue slots.
            b, P = st["b"], st["P"]
            X = Xs[b]
            Acp = op.tile([H, C], f32, tag="Acp")
            XA = op.tile([H, C], f32, tag="XA")
            Aps = []
            for j in range(2):
                Ap = ps.tile([H, 512], f32, tag="ps")
                nc.tensor.matmul(
                    Ap[:], st["YTs"][:], P[:, j * 512 : (j + 1) * 512],
                    start=True, stop=True,
                )
                Aps.append(Ap)
            nc.scalar.copy(Acp[:, 0:512], Aps[0][:])
            nc.vector.tensor_copy(Acp[:, 512:C], Aps[1][:])
            nc.gpsimd.tensor_mul(
                XA[:, 0:512], X[:, 0:512].bitcast(f32), Acp[:, 0:512]
            )
            nc.gpsimd.tensor_mul(
                XA[:, 512:C], X[:, 512:C].bitcast(f32), Acp[:, 512:C]
            )
            nc.sync.dma_start(out_t[b, 0], Acp[:])
            nc.sync.dma_start(out_t[b, 1], XA[:])

        def back_b1(st):
            # Deferred tt for batch b-1: runs early (deps are all previous-
            # cycle), so tts lands on ACT ahead of the PT copies and B never
            # waits long.
            b, P, XT, PT = st["b"], st["P"], st["XT"], st["PT"]
            r2 = st["r2"]
            # tt[:,128:256] = P @ X^T  (cols 0:128 accumulate junk, never
            # read; N=256 keeps the PE stream ahead of LDWEIGHTS)
            tt = ps2.tile([Q, 256], f32, tag="tt")
            for c in range(8):
                nc.tensor.matmul(
                    tt[:],
                    PT[:, c * 128 : (c + 1) * 128],
                    XT[:, c * 128 : c * 128 + 256],
                    start=(c == 0),
                    stop=(c == 7),
                )
            tts = pp.tile([Q, H], f32r, tag="tts")
            nc.scalar.activation(
                tts[:], tt[:, 128:256], IDENT, scale=r2[:]
            )
            st.update(tts=tts)

        def back_b2(st):
            b, P = st["b"], st["P"]
            tts = st["tts"]
            X = Xs[b]
            XB = op.tile([H, C], f32, tag="XB")

            BYPASS = mybir.AluOpType.bypass
            for j in range(2):
                Bps = ps.tile([H, 512], f32, tag="ps")
                nc.tensor.matmul(
                    Bps[:], tts[:], P[:, j * 512 : (j + 1) * 512],
                    start=True, stop=True,
                )
                nc.vector.scalar_tensor_tensor(
                    XB[:, j * 512 : (j + 1) * 512],
                    X[:, j * 512 : (j + 1) * 512].bitcast(f32),
                    1.0,
                    Bps[:],
                    BYPASS,
                    MULT,
                )
            nc.sync.dma_start(out_t[b, 2], XB[:])

        prev = None
        for b in range(BPC):
            st = front_a(b)
            if prev is not None:
                back_a(prev)
            front_b(b, st)
            mid_a(b, st)
            front_c(b, st)
            if prev is not None:
                back_b1(prev)
            mid_b(b, st)
            if prev is not None:
                back_b2(prev)
            front_d(b, st)
            prev = st
        back_a(prev)
        back_b1(prev)
        back_b2(prev)

    nc.compile()
    return nc


def kernel(context, question, w):
    global _NC, LAST_RESULTS
    from concourse import bass_utils

    if _NC is None:
        _NC = _build()

    context = np.ascontiguousarray(np.asarray(context), dtype=np.float32)
    question = np.ascontiguousarray(np.asarray(question), dtype=np.float32)
    w = np.ascontiguousarray(np.asarray(w), dtype=np.float32)

    in_maps = [
        {
            "context": context[c * BPC : (c + 1) * BPC],
            "question": question[c * BPC : (c + 1) * BPC],
            "w": w,
        }
        for c in range(NCORES)
    ]
    trace = bool(int(os.environ.get("KTRACE", "0")))
    LAST_RESULTS = bass_utils.run_bass_kernel_spmd(
        _NC, in_maps, core_ids=list(range(NCORES)), trace=trace
    )
    out = np.empty((B, 4 * H, C), dtype=np.float32)
    out[:, 0:H, :] = context
    for c in range(NCORES):
        res = LAST_RESULTS.results[c]["out"].reshape(BPC, 3 * H, C)
        out[c * BPC : (c + 1) * BPC, H:, :] = res
    return out


# revision 19
# speedup vs baseline: 1.1551x; 1.1551x over previous
"""CQAttention (BiDAF context-query attention) forward kernel for 8 Trainium2
NeuronCores.

Full inputs: context (64,128,1024) f32, question (64,128,128) f32, w (384,) f32.
Full output: (64, 512, 1024) f32.

Sharding: pure data parallel over batch — 8 batches per core, w replicated.

Math (per batch, X = context[b] (H,C), Y = question[b] (H,Q), w=(wq,wc,wcq)):
    Z   = wcq*Y + wc 1^T                      # (H,Q); wq term is softmax-invariant
    S^T = Z^T @ X                             # (Q,C) scores
    P   = exp(S^T)                            # unnormalized softmax numerators
    d   = rowsum(P); r = 1/d                  # softmax denominators (per q-row)
    A   = (diag(r) Y^T)^T @ P                 # = a^T                (H,C)
    tt  = P @ X^T                             # (Q,H)
    Bm  = (diag(r^2) tt)^T @ P                # = b^T = (s1 (s1^T c))^T  (H,C)
    out = [X; A; X*A; X*Bm]                   # (4H, C)

The run is HBM-bound (17.3 MB/core of DMA at ~400 GB/s ≈ 45 us floor + ~6.5 us
fixed framework preamble). Structure chosen to keep DMA saturated and the PE
stream gapless (so its clock ramps 1.2 -> 2.4 GHz):

- tt needs both P and X with the contraction dim (C) on partitions. Instead of
  16 PE transposes, 8 "combo" matmuls per batch compute, for each 128-chunk of
  C, X_chunk^T @ [I | Z] = [X^T_chunk | S_chunk] — the X^T chunk AND the
  scores in (C,Q) layout in one N=256 f32r pass. A second exp turns S_chunk
  into P^T directly. This removes the exp -> transpose serialization entirely:
  combo depends only on X and Z.
- 2-cycle software pipeline: cycle b runs {S, exp, denominators, combo, exp^T,
  XT copies} of batch b and {A, tt, B, output muls, output DMAs} of batch b-1,
  so every PE instruction depends only on previous-cycle products.
- All inputs prefetched up front (~12 us of DMA backlog); outputs (A, X*A,
  X*B) stream per batch. Copy/elementwise work is spread across ACT, DVE and
  Pool (Pool cannot touch PSUM, so it gets the SBUF-only X*A muls + Z).

All matmuls run in float32r (TF32-like, 1 cycle/row at N>=256). X/Y are DMA'd
as raw f32 bits into f32r tiles; engine-written f32r tiles (P, PT, XT, Z, tts)
are round-to-nearest by hardware. Elementwise consumers use .bitcast(f32).
"""

import os
import sys

import numpy as np

if "/opt/trn_rl_repo" not in sys.path:
    sys.path.insert(0, "/opt/trn_rl_repo")

B, H, C, Q = 64, 128, 1024, 128
NCORES = 8
BPC = B // NCORES  # batches per core


def _ensure_ntff_hook():
    """This container's `antenv` stub lacks `axon_hooks`, which
    bass_utils needs for NTFF profiling under axon (trace=True). Install
    a functional shadow module + register the ctypes-based hook."""
    import types

    try:
        from antenv.axon_hooks import get_axon_ntff_profile_hook  # noqa: F401

        return  # real module present
    except ImportError:
        pass
    try:
        import antenv

        mod = types.ModuleType("antenv.axon_hooks")
        _state = {"hook": None}

        def set_axon_ntff_profile_hook(h):
            _state["hook"] = h

        def get_axon_ntff_profile_hook():
            return _state["hook"]

        mod.set_axon_ntff_profile_hook = set_axon_ntff_profile_hook
        mod.get_axon_ntff_profile_hook = get_axon_ntff_profile_hook
        sys.modules["antenv.axon_hooks"] = mod
        antenv.axon_hooks = mod

        from trn_agent_boot.trn_boot import _ntff_profile_via_ctypes

        set_axon_ntff_profile_hook(
            _ntff_profile_via_ctypes("/opt/axon/libaxon_pjrt.so")
        )
    except Exception:
        pass  # profiling degrades; compute still works


_ensure_ntff_hook()

LAST_RESULTS = None
_NC = None


def _build():
    from contextlib import ExitStack

    import concourse.bacc as bacc
    import concourse.mybir as mybir
    import concourse.tile as tile
    from concourse import masks

    f32 = mybir.dt.float32
    f32r = mybir.dt.float32r
    bf16 = mybir.dt.bfloat16
    EXP = mybir.ActivationFunctionType.Exp
    IDENT = mybir.ActivationFunctionType.Identity
    MULT = mybir.AluOpType.mult
    ADD = mybir.AluOpType.add

    nc = bacc.Bacc(
        "TRN2", target_bir_lowering=False, debug=False, enable_asserts=False
    )
    ctx_t = nc.dram_tensor("context", (BPC, H, C), f32, kind="ExternalInput").ap()
    q_t = nc.dram_tensor("question", (BPC, H, Q), f32, kind="ExternalInput").ap()
    w_t = nc.dram_tensor("w", (3 * H,), f32, kind="ExternalInput").ap()
    # device writes blocks (A, X*A, X*B); block 0 == context is filled
    # host-side during unshard (pure passthrough of an input).
    out_t = nc.dram_tensor("out", (BPC, 3, H, C), f32, kind="ExternalOutput").ap()

    with tile.TileContext(nc) as tc, ExitStack() as ctx:
        const = ctx.enter_context(tc.tile_pool(name="const", bufs=1))
        xp = ctx.enter_context(tc.tile_pool(name="xp", bufs=BPC))
        yp = ctx.enter_context(tc.tile_pool(name="yp", bufs=BPC))
        pp = ctx.enter_context(tc.tile_pool(name="pp", bufs=2))
        op = ctx.enter_context(tc.tile_pool(name="op", bufs=3))
        ps = ctx.enter_context(tc.tile_pool(name="ps", bufs=6, space="PSUM"))
        ps2 = ctx.enter_context(tc.tile_pool(name="ps2", bufs=2, space="PSUM"))

        ident = const.tile([128, 128], f32, tag="ident")
        masks.make_identity(nc, ident[:])
        identr = const.tile([128, 128], f32r, tag="identr")
        nc.vector.tensor_copy(identr[:], ident[:])

        # w arrives as one contiguous (1,384) row (cheap single-descriptor
        # DMA); the (128,1) columns are produced by K=1 PE matmuls against
        # identity — avoids two slow 128-descriptor scatter DMAs at startup.
        w_row = const.tile([1, 3 * H], f32r, tag="w_row")
        nc.sync.dma_start(w_row[:], w_t.unsqueeze(0).bitcast(f32r))

        # Prefetch ALL inputs up front: ~12 us of guaranteed DMA backlog
        # while the compute pipeline fills. Batch 0's X is split so S can
        # start on the first half sooner.
        Xs = [xp.tile([H, C], f32r, tag="X", name=f"X{i}") for i in range(BPC)]
        Ys = [yp.tile([H, Q], f32r, tag="Y", name=f"Y{i}") for i in range(BPC)]
        nc.sync.dma_start(Xs[0][:, 0:512], ctx_t[0, :, 0:512].bitcast(f32r))
        nc.sync.dma_start(Ys[0][:], q_t[0].bitcast(f32r))
        nc.sync.dma_start(Xs[0][:, 512:C], ctx_t[0, :, 512:C].bitcast(f32r))
        for b in range(1, BPC):
            nc.sync.dma_start(Xs[b][:], ctx_t[b].bitcast(f32r))
            nc.sync.dma_start(Ys[b][:], q_t[b].bitcast(f32r))

        wps = ps.tile([128, 512], f32, tag="ps")
        nc.tensor.matmul(
            wps[:, 0:128], w_row[0:1, H : 2 * H], identr[0:1, 0:128],
            start=True, stop=True,
        )
        nc.tensor.matmul(
            wps[:, 128:256], w_row[0:1, 2 * H : 3 * H], identr[0:1, 0:128],
            start=True, stop=True,
        )
        wc = const.tile([128, 1], f32, tag="wc")
        wcq = const.tile([128, 1], f32, tag="wcq")
        nc.vector.tensor_copy(wc[:], wps[:, 0:1])
        nc.vector.tensor_copy(wcq[:], wps[:, 128:129])

        # Z = wcq*Y + wc, computed on Pool one cycle ahead so S never waits.
        zq0 = const.tile([H, Q], f32r, tag="zq0")
        zq1 = const.tile([H, Q], f32r, tag="zq1")
        zqs = [zq0, zq1]

        def make_Z(b):
            nc.gpsimd.tensor_scalar(
                zqs[b % 2][:], Ys[b][:].bitcast(f32), wcq[:], wc[:],
                MULT, ADD,
            )

        make_Z(0)

        def front_a(b):
            # S matmuls + first exp + yt: the PE ops every same-cycle ACT/DVE
            # dep hangs off, so they lead the cycle on all queues.
            Zt = zqs[b % 2][:]
            P = pp.tile([Q, C], f32r, tag="P")
            dh = pp.tile([Q, 2], f32, tag="dh")
            Shs = []
            for j in range(2):
                Sh = ps.tile([Q, 512], f32, tag="ps")
                nc.tensor.matmul(
                    Sh[:], Zt, Xs[b][:, j * 512 : (j + 1) * 512],
                    start=True, stop=True,
                )
                Shs.append(Sh)
            nc.scalar.activation(
                P[:, 0:512], Shs[0][:], EXP, accum_out=dh[:, 0:1]
            )
            yt = ps2.tile([128, 256], f32, tag="tt")
            nc.tensor.transpose(yt[:, 0:128].bitcast(f32r), Ys[b][:], identr[:])
            # XT: [pad | X^T] in bf16 for the tt matmuls — the pad block
            # keeps every N=256 tt window on initialized data (cols 0:128 of
            # each window accumulate junk, never read). YTs: diag(r) Y^T,
            # the A-matmul stationary operand.
            XT = pp.tile([128, 128 + C], bf16, tag="XT")
            nc.gpsimd.memset(XT[:, 0:128], 0)
            YTs = pp.tile([Q, H], f32r, tag="YTs")
            return dict(b=b, P=P, dh=dh, Shs=Shs, yt=yt, XT=XT, YTs=YTs)

        def front_b(b, st):
            # second exp (after back's A-copies on ACT)
            P, dh, Shs = st["P"], st["dh"], st["Shs"]
            nc.scalar.activation(
                P[:, 512:C], Shs[1][:], EXP, accum_out=dh[:, 1:2]
            )

        def front_c(b, st):
            # softmax denominators: dsum on Pool (SBUF-only), reciprocal on
            # DVE (only engine with it), YTs on DVE. r2 is computed late on
            # Pool — its consumer (tts) runs next cycle.
            dh, yt = st["dh"], st["yt"]
            dsum = pp.tile([Q, 1], f32, tag="dsum")
            nc.vector.tensor_add(dsum[:], dh[:, 0:1], dh[:, 1:2])
            rr = pp.tile([Q, 1], f32, tag="rr")
            nc.vector.reciprocal(rr[:], dsum[:])
            nc.vector.tensor_scalar_mul(st["YTs"][:], yt[:, 0:128], rr[:])
            st.update(rr=rr)

        def front_d(b, st):
            r2 = pp.tile([Q, 1], f32, tag="r2")
            nc.gpsimd.tensor_mul(r2[:], st["rr"][:], st["rr"][:])
            if b + 1 < BPC:
                make_Z(b + 1)
            st.update(r2=r2)

        def mid_a(b, st):
            # PE transposes of P and X into PSUM (streamed, ~85 ns each).
            # pt0-3 gate on exp0 only; xt chunks are dep-free fillers; the
            # first XT copy (DVE) drains xtps0 early for the PSUM ring.
            X, P, XT = Xs[b], st["P"], st["XT"]
            PT = pp.tile([128, C], bf16, tag="PT")
            ptps0 = ps.tile([128, 512], f32, tag="ps")
            for k in range(4):
                nc.tensor.transpose(
                    ptps0[:, k * 128 : (k + 1) * 128].bitcast(f32r),
                    P[:, k * 128 : (k + 1) * 128],
                    identr[:],
                )
            xtps = []
            for g in range(2):
                xg = ps.tile([128, 512], f32, tag="ps")
                for k in range(4):
                    c0 = g * 4 + k
                    nc.tensor.transpose(
                        xg[:, k * 128 : (k + 1) * 128].bitcast(f32r),
                        X[:, c0 * 128 : (c0 + 1) * 128],
                        identr[:],
                    )
                xtps.append(xg)
                if g == 0:
                    nc.vector.tensor_copy(XT[:, 128:640], xg[:])
            ptps1 = ps.tile([128, 512], f32, tag="ps")
            for k in range(4):
                nc.tensor.transpose(
                    ptps1[:, k * 128 : (k + 1) * 128].bitcast(f32r),
                    P[:, 512 + k * 128 : 512 + (k + 1) * 128],
                    identr[:],
                )
            st.update(PT=PT, ptps=(ptps0, ptps1), xtps1=xtps[1])

        def mid_b(b, st):
            XT, PT = st["XT"], st["PT"]
            ptps0, ptps1 = st["ptps"]
            nc.vector.tensor_copy(XT[:, 640 : 128 + C], st["xtps1"][:])
            nc.scalar.copy(PT[:, 0:512], ptps0[:])
            nc.scalar.copy(PT[:, 512:C], ptps1[:])

        def back_a(st):
            # Deferred A-section for batch b-1: A matmuls lead (all deps are
            # previous-cycle), copies/muls/DMAs follow on early queue slots.
            b, P = st["b"], st["P"]
            X = Xs[b]
            Acp = op.tile([H, C], f32, tag="Acp")
            XA = op.tile([H, C], f32, tag="XA")
            Aps = []
            for j in range(2):
                Ap = ps.tile([H, 512], f32, tag="ps")
                nc.tensor.matmul(
                    Ap[:], st["YTs"][:], P[:, j * 512 : (j + 1) * 512],
                    start=True, stop=True,
                )
                Aps.append(Ap)
            nc.scalar.copy(Acp[:, 0:512], Aps[0][:])
            nc.vector.tensor_copy(Acp[:, 512:C], Aps[1][:])
            nc.gpsimd.tensor_mul(
                XA[:, 0:512], X[:, 0:512].bitcast(f32), Acp[:, 0:512]
            )
            nc.gpsimd.tensor_mul(
                XA[:, 512:C], X[:, 512:C].bitcast(f32), Acp[:, 512:C]
            )
            nc.sync.dma_start(out_t[b, 0], Acp[:])
            nc.sync.dma_start(out_t[b, 1], XA[:])

        def back_b1(st):
            # Deferred tt for batch b-1: runs early (deps are all previous-
            # cycle), so tts lands on ACT ahead of the PT copies and B never
            # waits long.
            b, P, XT, PT = st["b"], st["P"], st["XT"], st["PT"]
            r2 = st["r2"]
            # tt[:,128:256] = P @ X^T  (cols 0:128 accumulate junk, never
            # read; N=256 keeps the PE stream ahead of LDWEIGHTS)
            tt = ps2.tile([Q, 256], f32, tag="tt")
            for c in range(8):
                nc.tensor.matmul(
                    tt[:],
                    PT[:, c * 128 : (c + 1) * 128],
                    XT[:, c * 128 : c * 128 + 256],
                    start=(c == 0),
                    stop=(c == 7),
                )
            tts = pp.tile([Q, H], f32r, tag="tts")
            nc.scalar.activation(
                tts[:], tt[:, 128:256], IDENT, scale=r2[:]
            )
            st.update(tts=tts)

        def back_b2(st):
            b, P = st["b"], st["P"]
            tts = st["tts"]
            X = Xs[b]
            XB = op.tile([H, C], f32, tag="XB")

            BYPASS = mybir.AluOpType.bypass
            for j in range(2):
                Bps = ps.tile([H, 512], f32, tag="ps")
                nc.tensor.matmul(
                    Bps[:], tts[:], P[:, j * 512 : (j + 1) * 512],
                    start=True, stop=True,
                )
                nc.vector.scalar_tensor_tensor(
                    XB[:, j * 512 : (j + 1) * 512],
                    X[:, j * 512 : (j + 1) * 512].bitcast(f32),
                    1.0,
                    Bps[:],
                    BYPASS,
                    MULT,
                )
            nc.sync.dma_start(out_t[b, 2], XB[:])

        prev = None
        for b in range(BPC):
            st = front_a(b)
            if prev is not None:
                back_a(prev)
            front_b(b, st)
            mid_a(b, st)
            front_c(b, st)
            if prev is not None:
                back_b1(prev)
            mid_b(b, st)
            if prev is not None:
                back_b2(prev)
            front_d(b, st)
            prev = st
        back_a(prev)
        back_b1(prev)
        back_b2(prev)

    nc.compile()
    return nc


def kernel(context, question, w):
    global _NC, LAST_RESULTS
    from concourse import bass_utils

    if _NC is None:
        _NC = _build()

    context = np.ascontiguousarray(np.asarray(context), dtype=np.float32)
    question = np.ascontiguousarray(np.asarray(question), dtype=np.float32)
    w = np.ascontiguousarray(np.asarray(w), dtype=np.float32)

    in_maps = [
        {
            "context": context[c * BPC : (c + 1) * BPC],
            "question": question[c * BPC : (c + 1) * BPC],
            "w": w,
        }
        for c in range(NCORES)
    ]
    trace = bool(int(os.environ.get("KTRACE", "0")))
    LAST_RESULTS = bass_utils.run_bass_kernel_spmd(
        _NC, in_maps, core_ids=list(range(NCORES)), trace=trace
    )
    out = np.empty((B, 4 * H, C), dtype=np.float32)
    out[:, 0:H, :] = context
    for c in range(NCORES):
        res = LAST_RESULTS.results[c]["out"].reshape(BPC, 3 * H, C)
        out[c * BPC : (c + 1) * BPC, H:, :] = res
    return out
